# revision 4
# baseline (speedup 1.0000x reference)
"""BPNet GNN message-passing kernel for 8 Trainium2 NeuronCores.

Strategy (forced by this image: no extended-GPSIMD ucode, no indirect DMA —
both crash the device, verified experimentally; only static DMA + PE + DVE/ACT
work):
  - Node-sharded output: core c owns nodes [512c, 512c+512). Every (edge,slot)
    pair is routed (host-side marshaling) to the owner core of its target node.
  - Host packs, per core, a position stream of pairs grouped by
    (node-chunk q in [0,4), slot i in [0,3)) -> 12 groups, each padded to whole
    128-position tiles. All per-pair data is baked into dense device tensors:
      xab  [128, L]  bf16 : one-hot-placed gathered node features + type
                            indicator for the pair's two COMPANION slots
                            (K-dim one-hot folds the per-edge weight selection
                            and bias into one fixed matmul).
      msel [128,T*64] bf16 : per-pair output-type selection mask (4x16 blocks)
      bsel [128,T*16] bf16 : per-pair pre-selected ho_bias row
      oneh [128,T*128]bf16 : per-tile one-hot incidence (lane -> node row)
  - Device: stage1 t = relu(W1^T @ xab) twice (companion a, b); fact = ta*tb;
    stage2 per tile: psum[128,64] = fact_tile^T @ W2cat(slot); select =
    (psum * msel) block-summed + bsel; segment-sum per tile:
    psum_nodes[:, 16q:+16] += oneh_tile^T @ msg_tile (PSUM accumulation).
  - Output per core: [128, 64] f32 = nodes [row, 16q+d]; unshard = pure
    reshape/concat on host. All arithmetic on the device.
"""

import numpy as np
import ml_dtypes

N, E, ORDER, D, RANK = 4096, 16384, 3, 13, 128
NP_ = ORDER + 1  # 4 types
NCORES = 8
NODES_PER_CORE = N // NCORES  # 512

bf16 = ml_dtypes.bfloat16

_COMPILED = {}  # Tg -> (nc, names)


def _build_program(Tg: int):
    import concourse.bacc as bacc
    import concourse.tile as tile
    from concourse import mybir

    T = 12 * Tg
    L = 128 * T
    NCH = L // 512

    nc = bacc.Bacc("TRN2", target_bir_lowering=False, debug=False,
                   num_devices=NCORES)
    BF, F32 = mybir.dt.bfloat16, mybir.dt.float32
    Relu = mybir.ActivationFunctionType.Relu
    mult = mybir.AluOpType.mult

    xab = nc.dram_tensor("xab", [128, L], BF, kind="ExternalInput").ap()
    w1a = nc.dram_tensor("w1a", [128, 128], BF, kind="ExternalInput").ap()
    w1b = nc.dram_tensor("w1b", [128, 128], BF, kind="ExternalInput").ap()
    w2 = nc.dram_tensor("w2", [128, 3, 64], BF, kind="ExternalInput").ap()
    msel = nc.dram_tensor("msel", [128, T // 4, 4, 64], BF, kind="ExternalInput").ap()
    bsel = nc.dram_tensor("bsel", [128, T // 4, 4, 16], BF, kind="ExternalInput").ap()
    oneh = nc.dram_tensor("oneh", [128, T * 128], BF, kind="ExternalInput").ap()
    out = nc.dram_tensor("out", [128, 64], F32, kind="ExternalOutput").ap()

    with tile.TileContext(nc) as tc:
        with tc.tile_pool(name="inp", bufs=1) as inp, \
             tc.tile_pool(name="work", bufs=1) as work, \
             tc.tile_pool(name="ps1", bufs=2, space="PSUM") as ps1, \
             tc.tile_pool(name="ps2", bufs=2, space="PSUM") as ps2, \
             tc.tile_pool(name="psn", bufs=1, space="PSUM") as psn:
            xab_sb = inp.tile([128, L], BF, tag="xab")
            w1a_sb = inp.tile([128, 128], BF, tag="w1a")
            w1b_sb = inp.tile([128, 128], BF, tag="w1b")
            w2_sb = inp.tile([128, 3, 64], BF, tag="w2")
            msel_sb = inp.tile([128, T // 4, 4, 64], BF, tag="msel")
            bsel_sb = inp.tile([128, T // 4, 4, 16], BF, tag="bsel")
            oneh_sb = inp.tile([128, T * 128], BF, tag="oneh")

            nc.sync.dma_start(xab_sb[:], xab[:])
            nc.sync.dma_start(w1a_sb[:], w1a[:])
            nc.sync.dma_start(w1b_sb[:], w1b[:])
            nc.sync.dma_start(w2_sb[:, :, :], w2[:, :, :])
            nc.sync.dma_start(msel_sb[:, :, :, :], msel[:, :, :, :])
            nc.sync.dma_start(bsel_sb[:, :, :, :], bsel[:, :, :, :])
            nc.sync.dma_start(oneh_sb[:], oneh[:])

            ta = work.tile([128, L], BF, tag="ta")
            tb = work.tile([128, L], BF, tag="tb")
            fact = work.tile([128, L], BF, tag="fact")
            msg = work.tile([128, T // 4, 4, 16], BF, tag="msg")
            sel = work.tile([128, 4, 64], F32, tag="sel")
            f1 = work.tile([128, 4, 32], F32, tag="f1")
            f2 = work.tile([128, 4, 16], F32, tag="f2")
            oc = work.tile([128, 64], F32, tag="oc")

            # stage 1: t = relu(W1^T @ xab), companions a and b
            for w_sb, tdst in ((w1a_sb, ta), (w1b_sb, tb)):
                for ch in range(NCH):
                    p = ps1.tile([128, 512], F32, tag="p1")
                    sl = slice(512 * ch, 512 * (ch + 1))
                    nc.tensor.matmul(p[:], w_sb[:], xab_sb[:, sl],
                                     start=True, stop=True)
                    if ch % 2 == 0:
                        nc.vector.tensor_scalar_max(tdst[:, sl], p[:], 0.0)
                    else:
                        nc.scalar.activation(tdst[:, sl], p[:], Relu)

            # fact = ta * tb  (L is always a multiple of 1536)
            for ch in range(L // 1536):
                sl = slice(1536 * ch, 1536 * (ch + 1))
                nc.vector.tensor_mul(fact[:, sl], ta[:, sl], tb[:, sl])

            # stage 2 + select, superblocks of 4 tiles
            for s in range(T // 4):
                pb = ps2.tile([128, 4, 64], F32, tag="p2")
                for k in range(4):
                    t = 4 * s + k
                    i_slot = (t // Tg) % 3
                    nc.tensor.matmul(
                        pb[:, k, :],
                        fact[:, 128 * t:128 * (t + 1)],
                        w2_sb[:, i_slot, :],
                        start=True, stop=True,
                    )
                nc.vector.tensor_tensor(
                    sel[:, :, :], pb[:, :, :],
                    msel_sb[:, s, :, :], mult)
                nc.vector.tensor_add(f1[:, :, :], sel[:, :, 0:32],
                                     sel[:, :, 32:64])
                nc.vector.tensor_add(f2[:, :, :], f1[:, :, 0:16],
                                     f1[:, :, 16:32])
                nc.vector.tensor_add(msg[:, s, :, :], f2[:, :, :],
                                     bsel_sb[:, s, :, :])

            # segment sum: psum_nodes[:, 16q:+16] += oneh_t^T @ msg_t
            pn = psn.tile([128, 64], F32, tag="pn")
            for t in range(T):
                q = t // (3 * Tg)
                first = (t % (3 * Tg)) == 0
                last = (t % (3 * Tg)) == 3 * Tg - 1
                nc.tensor.matmul(
                    pn[:, 16 * q:16 * (q + 1)],
                    oneh_sb[:, 128 * t:128 * (t + 1)],
                    msg[:, t // 4, t % 4, :],
                    start=first, stop=last,
                )

            nc.vector.tensor_copy(oc[:], pn[:])
            nc.sync.dma_start(out[:], oc[:])

    nc.compile()
    return nc


def _prep_inputs(nodes, bp_params, bp_bias, ho_params, ho_bias, edges,
                 edge_types):
    nodes = np.asarray(nodes, np.float32)
    bp_params = np.asarray(bp_params, np.float32)
    bp_bias = np.asarray(bp_bias, np.float32)
    ho_params = np.asarray(ho_params, np.float32)
    ho_bias = np.asarray(ho_bias, np.float32)
    edges = np.asarray(edges, np.int64)
    edge_types = np.asarray(edge_types, np.int64)

    nodes_b = nodes.astype(bf16)

    # group pairs: (core, q, i) ; pair list per group
    tgt = edges  # [E, 3]
    owner = tgt // NODES_PER_CORE
    q = (tgt % NODES_PER_CORE) // 128
    r = tgt % 128

    group_lists = {}
    maxcount = 1
    for c in range(NCORES):
        for i in range(ORDER):
            sel_c = owner[:, i] == c
            for qq in range(4):
                es = np.nonzero(sel_c & (q[:, i] == qq))[0]
                group_lists[(c, qq, i)] = es
                maxcount = max(maxcount, len(es))
    Tg = (maxcount + 127) // 128
    T = 12 * Tg
    L = 128 * T

    # weight tables (shared across cores)
    w1a = np.zeros((128, 128), np.float32)
    w1b = np.zeros((128, 128), np.float32)
    for p in range(NP_):
        w1a[13 * p:13 * p + 13, :] = bp_params[p]
        w1a[52 + p, :] = bp_bias[p, 0, :]
        w1b[64 + 13 * p:64 + 13 * p + 13, :] = bp_params[p]
        w1b[116 + p, :] = bp_bias[p, 0, :]
    w2 = np.zeros((128, 3, 64), np.float32)
    for i in range(ORDER):
        for p in range(NP_):
            w2[:, i, 16 * p:16 * p + 13] = ho_params[i, p]

    in_maps = []
    for c in range(NCORES):
        xab = np.zeros((128, L), np.float32)
        msel = np.zeros((128, T * 64), np.float32)
        bsel_a = np.zeros((128, T * 16), np.float32)
        oneh = np.zeros((128, T * 128), np.float32)
        for qq in range(4):
            for i in range(ORDER):
                es = group_lists[(c, qq, i)]
                g = qq * 3 + i
                k = np.arange(len(es))
                x = 128 * g * Tg + k
                t_arr = x // 128
                lane = x % 128
                a, b = (i + 1) % 3, (i + 2) % 3
                ta_t = edge_types[es, a]
                tb_t = edge_types[es, b]
                fa = nodes_b[edges[es, a]].astype(np.float32)  # [m, 13]
                fb = nodes_b[edges[es, b]].astype(np.float32)
                for dd in range(D):
                    xab[13 * ta_t + dd, x] = fa[:, dd]
                    xab[64 + 13 * tb_t + dd, x] = fb[:, dd]
                xab[52 + ta_t, x] = 1.0
                xab[116 + tb_t, x] = 1.0
                p_e = edge_types[es, i]
                for dd in range(D):
                    msel[lane, 64 * t_arr + 16 * p_e + dd] = 1.0
                    bsel_a[lane, 16 * t_arr + dd] = ho_bias[i, p_e, 0, dd]
                oneh[lane, 128 * t_arr + r[es, i]] = 1.0
        in_maps.append({
            "xab": xab.astype(bf16),
            "w1a": w1a.astype(bf16),
            "w1b": w1b.astype(bf16),
            "w2": w2.astype(bf16),
            "msel": msel.reshape(128, T // 4, 4, 64).astype(bf16),
            "bsel": bsel_a.reshape(128, T // 4, 4, 16).astype(bf16),
            "oneh": oneh.astype(bf16),
        })
    return in_maps, Tg


def kernel(nodes, bp_params, bp_bias, ho_params, ho_bias, edges, edge_types,
           atoms=None, atom_edges=None, _run_kwargs=None):
    from concourse.bass_utils import run_bass_kernel_spmd

    in_maps, Tg = _prep_inputs(nodes, bp_params, bp_bias, ho_params, ho_bias,
                               edges, edge_types)
    if Tg not in _COMPILED:
        _COMPILED[Tg] = _build_program(Tg)
    nc = _COMPILED[Tg]

    res = run_bass_kernel_spmd(nc, in_maps, core_ids=list(range(NCORES)),
                               **(_run_kwargs or {}))
    full = np.zeros((N, D), np.float32)
    for c in range(NCORES):
        oc = res.results[c]["out"]  # [128, 64]
        for qq in range(4):
            full[NODES_PER_CORE * c + 128 * qq:
                 NODES_PER_CORE * c + 128 * (qq + 1), :] = oc[:, 16 * qq:
                                                             16 * qq + 13]
    kernel._last_result = res
    return full


# revision 5
# speedup vs baseline: 1.0153x; 1.0153x over previous
"""BPNet GNN message-passing kernel for 8 Trainium2 NeuronCores.

Strategy (forced by this image: no extended-GPSIMD ucode, no indirect DMA —
both crash the device, verified experimentally; only static DMA + PE + DVE/ACT
work):
  - Node-sharded output: core c owns nodes [512c, 512c+512). Every (edge,slot)
    pair is routed (host-side marshaling) to the owner core of its target node.
  - Host packs, per core, a position stream of pairs grouped by
    (node-chunk q in [0,4), slot i in [0,3)) -> 12 groups, each padded to whole
    128-position tiles. All per-pair data is baked into dense device tensors:
      xab  [128, L]  bf16 : one-hot-placed gathered node features + type
                            indicator for the pair's two COMPANION slots
                            (K-dim one-hot folds the per-edge weight selection
                            and bias into one fixed matmul).
      msel [128,T*64] bf16 : per-pair output-type selection mask (4x16 blocks)
      bsel [128,T*16] bf16 : per-pair pre-selected ho_bias row
      oneh [128,T*128]bf16 : per-tile one-hot incidence (lane -> node row)
  - Device: stage1 t = relu(W1^T @ xab) twice (companion a, b); fact = ta*tb;
    stage2 per tile: psum[128,64] = fact_tile^T @ W2cat(slot); select =
    (psum * msel) block-summed + bsel; segment-sum per tile:
    psum_nodes[:, 16q:+16] += oneh_tile^T @ msg_tile (PSUM accumulation).
  - Output per core: [128, 64] f32 = nodes [row, 16q+d]; unshard = pure
    reshape/concat on host. All arithmetic on the device.
"""

import numpy as np
import ml_dtypes

N, E, ORDER, D, RANK = 4096, 16384, 3, 13, 128
NP_ = ORDER + 1  # 4 types
NCORES = 8
NODES_PER_CORE = N // NCORES  # 512

bf16 = ml_dtypes.bfloat16

_COMPILED = {}  # Tg -> (nc, names)


def _build_program(Tg: int):
    import concourse.bacc as bacc
    import concourse.tile as tile
    from concourse import mybir

    T = 12 * Tg
    L = 128 * T
    NCH = L // 512

    nc = bacc.Bacc("TRN2", target_bir_lowering=False, debug=False,
                   num_devices=NCORES)
    BF, F32 = mybir.dt.bfloat16, mybir.dt.float32
    Relu = mybir.ActivationFunctionType.Relu
    Copy = mybir.ActivationFunctionType.Copy
    mult = mybir.AluOpType.mult

    xab = nc.dram_tensor("xab", [128, L], BF, kind="ExternalInput").ap()
    w1a = nc.dram_tensor("w1a", [128, 128], BF, kind="ExternalInput").ap()
    w1b = nc.dram_tensor("w1b", [128, 128], BF, kind="ExternalInput").ap()
    w2 = nc.dram_tensor("w2", [128, 3, 64], BF, kind="ExternalInput").ap()
    msel = nc.dram_tensor("msel", [128, T // 4, 4, 64], BF, kind="ExternalInput").ap()
    bsel = nc.dram_tensor("bsel", [128, T // 4, 4, 16], BF, kind="ExternalInput").ap()
    oneh = nc.dram_tensor("oneh", [128, T * 128], BF, kind="ExternalInput").ap()
    out = nc.dram_tensor("out", [128, 64], F32, kind="ExternalOutput").ap()

    with tile.TileContext(nc) as tc:
        with tc.tile_pool(name="inp", bufs=1) as inp, \
             tc.tile_pool(name="work", bufs=1) as work, \
             tc.tile_pool(name="ps1", bufs=2, space="PSUM") as ps1, \
             tc.tile_pool(name="ps2", bufs=2, space="PSUM") as ps2, \
             tc.tile_pool(name="psn", bufs=1, space="PSUM") as psn:
            xab_sb = inp.tile([128, L], BF, tag="xab")
            w1a_sb = inp.tile([128, 128], BF, tag="w1a")
            w1b_sb = inp.tile([128, 128], BF, tag="w1b")
            w2_sb = inp.tile([128, 3, 64], BF, tag="w2")
            msel_sb = inp.tile([128, T // 4, 4, 64], BF, tag="msel")
            bsel_sb = inp.tile([128, T // 4, 4, 16], BF, tag="bsel")
            oneh_sb = inp.tile([128, T * 128], BF, tag="oneh")

            nc.sync.dma_start(xab_sb[:], xab[:])
            nc.sync.dma_start(w1a_sb[:], w1a[:])
            nc.sync.dma_start(w1b_sb[:], w1b[:])
            nc.sync.dma_start(w2_sb[:, :, :], w2[:, :, :])
            nc.sync.dma_start(msel_sb[:, :, :, :], msel[:, :, :, :])
            nc.sync.dma_start(bsel_sb[:, :, :, :], bsel[:, :, :, :])
            nc.sync.dma_start(oneh_sb[:], oneh[:])

            ta = work.tile([128, L], BF, tag="ta")
            tb = work.tile([128, L], BF, tag="tb")
            fact = work.tile([128, L], BF, tag="fact")
            msg = work.tile([128, T // 4, 4, 16], BF, tag="msg")
            sel = work.tile([128, 4, 64], BF, tag="sel")
            f1 = work.tile([128, 4, 32], BF, tag="f1")
            f2 = work.tile([128, 4, 16], BF, tag="f2")
            oc = work.tile([128, 64], F32, tag="oc")

            # stage 1: t = relu(W1^T @ xab), companions a and b
            for w_sb, tdst in ((w1a_sb, ta), (w1b_sb, tb)):
                for ch in range(NCH):
                    p = ps1.tile([128, 512], F32, tag="p1")
                    sl = slice(512 * ch, 512 * (ch + 1))
                    nc.tensor.matmul(p[:], w_sb[:], xab_sb[:, sl],
                                     start=True, stop=True)
                    if ch % 2 == 0:
                        nc.vector.tensor_scalar_max(tdst[:, sl], p[:], 0.0)
                    else:
                        nc.scalar.activation(tdst[:, sl], p[:], Relu)

            # fact = ta * tb  (L is always a multiple of 1536)
            for ch in range(L // 1536):
                sl = slice(1536 * ch, 1536 * (ch + 1))
                nc.vector.tensor_mul(fact[:, sl], ta[:, sl], tb[:, sl])

            # stage 2 + select, superblocks of 4 tiles.
            # ScalarE copies PSUM->SBUF bf16; DVE select chain runs bf16 2x.
            for s in range(T // 4):
                pb = ps2.tile([128, 4, 64], F32, tag="p2")
                for k in range(4):
                    t = 4 * s + k
                    i_slot = (t // Tg) % 3
                    nc.tensor.matmul(
                        pb[:, k, :],
                        fact[:, 128 * t:128 * (t + 1)],
                        w2_sb[:, i_slot, :],
                        start=True, stop=True,
                    )
                pc = work.tile([128, 4, 64], BF, tag="pc")
                nc.scalar.activation(pc[:, :, :], pb[:, :, :], Copy)
                nc.vector.tensor_tensor(
                    sel[:, :, :], pc[:, :, :],
                    msel_sb[:, s, :, :], mult)
                nc.vector.tensor_add(f1[:, :, :], sel[:, :, 0:32],
                                     sel[:, :, 32:64])
                nc.vector.tensor_add(f2[:, :, :], f1[:, :, 0:16],
                                     f1[:, :, 16:32])
                nc.vector.tensor_add(msg[:, s, :, :], f2[:, :, :],
                                     bsel_sb[:, s, :, :])

            # segment sum: psum_nodes[:, 16q:+16] += oneh_t^T @ msg_t
            pn = psn.tile([128, 64], F32, tag="pn")
            for t in range(T):
                q = t // (3 * Tg)
                first = (t % (3 * Tg)) == 0
                last = (t % (3 * Tg)) == 3 * Tg - 1
                nc.tensor.matmul(
                    pn[:, 16 * q:16 * (q + 1)],
                    oneh_sb[:, 128 * t:128 * (t + 1)],
                    msg[:, t // 4, t % 4, :],
                    start=first, stop=last,
                )

            nc.vector.tensor_copy(oc[:], pn[:])
            nc.sync.dma_start(out[:], oc[:])

    nc.compile()
    return nc


def _prep_inputs(nodes, bp_params, bp_bias, ho_params, ho_bias, edges,
                 edge_types):
    nodes = np.asarray(nodes, np.float32)
    bp_params = np.asarray(bp_params, np.float32)
    bp_bias = np.asarray(bp_bias, np.float32)
    ho_params = np.asarray(ho_params, np.float32)
    ho_bias = np.asarray(ho_bias, np.float32)
    edges = np.asarray(edges, np.int64)
    edge_types = np.asarray(edge_types, np.int64)

    nodes_b = nodes.astype(bf16)

    # group pairs: (core, q, i) ; pair list per group
    tgt = edges  # [E, 3]
    owner = tgt // NODES_PER_CORE
    q = (tgt % NODES_PER_CORE) // 128
    r = tgt % 128

    group_lists = {}
    maxcount = 1
    for c in range(NCORES):
        for i in range(ORDER):
            sel_c = owner[:, i] == c
            for qq in range(4):
                es = np.nonzero(sel_c & (q[:, i] == qq))[0]
                group_lists[(c, qq, i)] = es
                maxcount = max(maxcount, len(es))
    Tg = (maxcount + 127) // 128
    T = 12 * Tg
    L = 128 * T

    # weight tables (shared across cores)
    w1a = np.zeros((128, 128), np.float32)
    w1b = np.zeros((128, 128), np.float32)
    for p in range(NP_):
        w1a[13 * p:13 * p + 13, :] = bp_params[p]
        w1a[52 + p, :] = bp_bias[p, 0, :]
        w1b[64 + 13 * p:64 + 13 * p + 13, :] = bp_params[p]
        w1b[116 + p, :] = bp_bias[p, 0, :]
    w2 = np.zeros((128, 3, 64), np.float32)
    for i in range(ORDER):
        for p in range(NP_):
            w2[:, i, 16 * p:16 * p + 13] = ho_params[i, p]

    in_maps = []
    for c in range(NCORES):
        xab = np.zeros((128, L), np.float32)
        msel = np.zeros((128, T * 64), np.float32)
        bsel_a = np.zeros((128, T * 16), np.float32)
        oneh = np.zeros((128, T * 128), np.float32)
        for qq in range(4):
            for i in range(ORDER):
                es = group_lists[(c, qq, i)]
                g = qq * 3 + i
                k = np.arange(len(es))
                x = 128 * g * Tg + k
                t_arr = x // 128
                lane = x % 128
                a, b = (i + 1) % 3, (i + 2) % 3
                ta_t = edge_types[es, a]
                tb_t = edge_types[es, b]
                fa = nodes_b[edges[es, a]].astype(np.float32)  # [m, 13]
                fb = nodes_b[edges[es, b]].astype(np.float32)
                for dd in range(D):
                    xab[13 * ta_t + dd, x] = fa[:, dd]
                    xab[64 + 13 * tb_t + dd, x] = fb[:, dd]
                xab[52 + ta_t, x] = 1.0
                xab[116 + tb_t, x] = 1.0
                p_e = edge_types[es, i]
                for dd in range(D):
                    msel[lane, 64 * t_arr + 16 * p_e + dd] = 1.0
                    bsel_a[lane, 16 * t_arr + dd] = ho_bias[i, p_e, 0, dd]
                oneh[lane, 128 * t_arr + r[es, i]] = 1.0
        in_maps.append({
            "xab": xab.astype(bf16),
            "w1a": w1a.astype(bf16),
            "w1b": w1b.astype(bf16),
            "w2": w2.astype(bf16),
            "msel": msel.reshape(128, T // 4, 4, 64).astype(bf16),
            "bsel": bsel_a.reshape(128, T // 4, 4, 16).astype(bf16),
            "oneh": oneh.astype(bf16),
        })
    return in_maps, Tg


def kernel(nodes, bp_params, bp_bias, ho_params, ho_bias, edges, edge_types,
           atoms=None, atom_edges=None, _run_kwargs=None):
    from concourse.bass_utils import run_bass_kernel_spmd

    in_maps, Tg = _prep_inputs(nodes, bp_params, bp_bias, ho_params, ho_bias,
                               edges, edge_types)
    if Tg not in _COMPILED:
        _COMPILED[Tg] = _build_program(Tg)
    nc = _COMPILED[Tg]

    res = run_bass_kernel_spmd(nc, in_maps, core_ids=list(range(NCORES)),
                               **(_run_kwargs or {}))
    full = np.zeros((N, D), np.float32)
    for c in range(NCORES):
        oc = res.results[c]["out"]  # [128, 64]
        for qq in range(4):
            full[NODES_PER_CORE * c + 128 * qq:
                 NODES_PER_CORE * c + 128 * (qq + 1), :] = oc[:, 16 * qq:
                                                             16 * qq + 13]
    kernel._last_result = res
    return full
